# revision 8
# baseline (speedup 1.0000x reference)
"""NTN kernel: f16 stream, bias folded into PSUM, ACT-engine relu.

y = relu(x1 @ M + c) @ u,  M = V[:,:D] + (W @ x2)^T  (128x16),
c = x2 @ V[:,D:]^T + b,    u = U[:,0].

Everything affine is folded into the matmul / host prep:

    u_k relu(z_k + c_k) = s_k * relu(|u_k| z_k + |u_k| c_k),  s = sign(u)

Host prep scales M's columns by |u| (M2 = M diag|u|, f16) and permutes
positive-u columns first (KP of them).  On device, per 64-tile group:

    PE:   one "ones-row" matmul seeds the PSUM group with the c2 row
          broadcast (bias add for free), then one 128x128 @ 128x16 f16
          matmul per row-tile accumulates z.
    ACT:  relu straight out of PSUM into an f16 elem tile (this also
          releases the PSUM bank -- ACT is otherwise idle, so PSUM
          recycling never waits on the DVE).
    DVE:  plain-sum reduce over the first KP columns -> y, reduce over
          the rest -> rn, y -= rn.  (f16 input = 2x DVE throughput.)
    Pool: tiny param DMAs at start, y stored in 2-group slabs.
    SP:   all x DMA issues (single queue saturates the 16 DMA engines).

x1 streams as f16 (2 B/elem -> 16 MB/core, rel err ~5e-4 vs tolerance
2e-2).  KP depends on the sign pattern of u, so the program is built
after inputs arrive (cached per KP).  The final partial chunk is
split small so almost nothing trails the last DMA byte.
"""

import numpy as np

import concourse.bass as bass
import concourse.bacc as bacc
import concourse.mybir as mybir
import concourse.tile as tile

N, D, K = 500000, 128, 16
NCORES = 8
ROWS_PER_CORE = N // NCORES
TILES = 489
RPC = TILES * 128
GROUP = 64
DMA_CHUNK = 64
F32 = mybir.dt.float32
F16 = mybir.dt.float16


def _build_program(kp):
    nc = bacc.Bacc(None, target_bir_lowering=False)

    xh = nc.dram_tensor("xh", [128, RPC], F16, kind="ExternalInput")
    mt = nc.dram_tensor("mt", [128, K], F16, kind="ExternalInput")
    ones = nc.dram_tensor("ones", [1, 128], F16, kind="ExternalInput")
    crow = nc.dram_tensor("crow", [1, GROUP, K], F16, kind="ExternalInput")
    y = nc.dram_tensor("y", [128, TILES], F32, kind="ExternalOutput")

    with tile.TileContext(nc) as tc:
        with (
            tc.tile_pool(name="singles", bufs=1) as singles,
            tc.tile_pool(name="xin", bufs=6) as xin,
            tc.tile_pool(name="zp", bufs=3, space="PSUM") as zpool,
            tc.tile_pool(name="work", bufs=4) as work,
            tc.tile_pool(name="yout", bufs=1) as yout,
        ):
            sizes = []
            rem = TILES
            while rem > DMA_CHUNK:
                sizes.append(DMA_CHUNK)
                rem -= DMA_CHUNK
            while rem > 16:
                sizes.append(16)
                rem -= 16
            sizes.append(rem)

            chunk_tiles = []
            c0 = 0
            for i, nct in enumerate(sizes):
                xh_t = xin.tile([128, DMA_CHUNK * 128], F16, tag="xh")
                nc.sync.dma_start(
                    xh_t[:, : nct * 128], xh[:, c0 * 128 : (c0 + nct) * 128]
                )
                chunk_tiles.append((c0, nct, xh_t))
                c0 += nct
            assert c0 == TILES

            mt_sb = singles.tile([128, K], F16)
            nc.gpsimd.dma_start(mt_sb, mt[:, :])
            ones_sb = singles.tile([1, 128], F16)
            nc.gpsimd.dma_start(ones_sb, ones[:, :])
            crow_sb = singles.tile([1, GROUP, K], F16)
            nc.gpsimd.dma_start(crow_sb, crow[:, :, :])

            y_sb = yout.tile([128, TILES], F32)

            stored = 0
            reduced = 0
            done_groups = 0

            def emit_ystore():
                nonlocal stored
                if reduced > stored:
                    nc.gpsimd.dma_start(
                        y[:, stored:reduced], y_sb[:, stored:reduced]
                    )
                    stored = reduced

            for c0, nct, xh_t in chunk_tiles:
                g0 = 0
                while g0 < nct:
                    nt = min(GROUP, nct - g0)
                    t0 = c0 + g0
                    zp = zpool.tile([128, GROUP, K], F32, tag="z")
                    # Bias: zp[:, t, k] = c2[k] for all t, via ones-row
                    # matmul.  PSUM banks are 512 f32 per partition, so
                    # seed in 32-tile (one-bank) pieces.
                    for b0 in range(0, nt, 32):
                        b1 = min(b0 + 32, nt)
                        nc.tensor.matmul(
                            zp[:, b0:b1, :], ones_sb[:, :],
                            crow_sb[:, b0:b1, :], start=True, stop=False,
                        )
                    for t in range(nt):
                        sl = slice((g0 + t) * 128, (g0 + t + 1) * 128)
                        nc.tensor.matmul(
                            zp[:, t, :], xh_t[:, sl], mt_sb[:, :],
                            start=False, stop=True,
                        )
                    elem = work.tile([128, GROUP, K], F16, tag="elem")
                    nc.scalar.activation(
                        elem[:, :nt, :], zp[:, :nt, :],
                        mybir.ActivationFunctionType.Relu,
                    )
                    ysl = y_sb[:, t0 : t0 + nt]
                    if kp > 0:
                        nc.vector.tensor_reduce(
                            ysl, elem[:, :nt, :kp],
                            axis=mybir.AxisListType.X, op=mybir.AluOpType.add,
                        )
                    if kp < K:
                        rn = work.tile([128, GROUP], F32, tag="rn")
                        nc.vector.tensor_reduce(
                            rn[:, :nt], elem[:, :nt, kp:],
                            axis=mybir.AxisListType.X, op=mybir.AluOpType.add,
                        )
                        if kp > 0:
                            nc.vector.tensor_tensor(
                                ysl, ysl, rn[:, :nt],
                                op=mybir.AluOpType.subtract,
                            )
                        else:
                            nc.vector.tensor_scalar(
                                ysl, rn[:, :nt], -1.0,
                                op=mybir.AluOpType.mult,
                            )
                    reduced = t0 + nt
                    done_groups += 1
                    if done_groups % 2 == 0:
                        emit_ystore()
                    g0 += nt

            emit_ystore()

    nc.compile()
    return nc


_NC_CACHE = {}


def _get_program(kp):
    if kp not in _NC_CACHE:
        _NC_CACHE[kp] = _build_program(kp)
    return _NC_CACHE[kp]


def _host_prep(x1, x2, V, W, b, U):
    x1 = np.asarray(x1, dtype=np.float32)
    x2 = np.asarray(x2, dtype=np.float64)
    V = np.asarray(V, dtype=np.float64)
    W = np.asarray(W, dtype=np.float64)
    b = np.asarray(b, dtype=np.float64)
    U = np.asarray(U, dtype=np.float64)

    M = V[:, :D] + np.einsum("kde,e->kd", W, x2[0])   # (K, D)
    c = (x2[0] @ V[:, D:].T) + b                      # (K,)
    u = U[:, 0]                                       # (K,)

    # Positive-u columns first; fold |u| into M and c so the epilogue
    # is relu / plain sums / subtract.
    perm = np.argsort(u <= 0, kind="stable")
    kp = int(np.sum(u > 0))
    up = np.abs(u[perm])
    M2 = M[perm] * up[:, None]                        # (K, D)
    c2 = c[perm] * up                                 # (K,)

    mt = np.ascontiguousarray(M2.T.astype(np.float16))
    ones = np.ones((1, 128), dtype=np.float16)
    crow = np.ascontiguousarray(
        np.broadcast_to(c2.astype(np.float16), (1, GROUP, K))
    )

    in_maps = []
    for cidx in range(NCORES):
        sl = x1[cidx * ROWS_PER_CORE : (cidx + 1) * ROWS_PER_CORE]
        hbuf = np.zeros((128, RPC), dtype=np.float16)
        hbuf[:, :ROWS_PER_CORE] = sl.T.astype(np.float16)
        in_maps.append({"xh": hbuf, "mt": mt, "ones": ones, "crow": crow})
    return in_maps, kp


def _gather(results):
    outs = []
    for cidx in range(NCORES):
        yc = np.asarray(results[cidx]["y"])
        outs.append(yc.T.reshape(-1)[:ROWS_PER_CORE])
    return np.concatenate(outs).reshape(N, 1).astype(np.float32)


def run_device(in_maps, kp, trace=False):
    from concourse.bass_utils import run_bass_kernel_spmd

    nc = _get_program(kp)
    res = run_bass_kernel_spmd(
        nc, in_maps, core_ids=list(range(NCORES)), trace=trace
    )
    return res


def kernel(x1, x2, V, W, b, U):
    in_maps, kp = _host_prep(x1, x2, V, W, b, U)
    res = run_device(in_maps, kp, trace=False)
    return _gather(res.results)


# revision 9
# speedup vs baseline: 1.0467x; 1.0467x over previous
"""NTN kernel: f16 stream with the bias folded into the input encoding.

y = relu(x1 @ M + c) @ u,  M = V[:,:D] + (W @ x2)^T  (128x16),
c = x2 @ V[:,D:]^T + b,    u = U[:,0].

Three host-side folds make the device program trivial:

1. u into M:   u_k relu(z_k + c_k) = s_k relu(|u_k| z_k + |u_k| c_k),
   so M2 = M diag|u| with positive-u columns permuted first (KP of
   them) and y = sum(first KP) - sum(rest).
2. c into x:   pick the min-norm row offset D with D @ M2 = c2
   (pinv of the well-conditioned random 128x16 M2; ||D|| ~ 0.3).
   Streaming x~ = x1 + D makes the matmul produce z + c directly --
   no bias matmul, no threshold tables on device.
3. f16 encode: x~ and M2 stream as f16 (2 B/elem -> 16 MB/core,
   rel err ~5e-4 vs tolerance 2e-2).

Device, per 64-tile group (one DMA chunk):
    PE:   one 128x128 @ 128x16 f16 matmul per row-tile into PSUM
    ACT:  relu straight out of PSUM into an f16 elem tile (also
          releases the PSUM bank; ACT is otherwise idle)
    DVE:  plain-sum reduce over first KP columns -> y, reduce over
          the rest -> rn, y -= rn   (f16 input = 2x DVE throughput)
    Pool: y stored in 2-group slabs
    SP:   mt param DMA first (so it can't be starved by the stream),
          then all x chunk DMAs on the single sync queue.

Engine loads per 2.44us group cadence: PE 1.7us, ACT 1.2, DVE 1.0,
Pool 0.4 -- nothing rate-limits but the DMA stream itself.  KP depends
on the sign pattern of u, so the program is built after inputs arrive
(cached per KP).  The final partial chunk is split small so almost
nothing trails the last DMA byte.
"""

import numpy as np

import concourse.bass as bass
import concourse.bacc as bacc
import concourse.mybir as mybir
import concourse.tile as tile

N, D, K = 500000, 128, 16
NCORES = 8
ROWS_PER_CORE = N // NCORES
TILES = 489
RPC = TILES * 128
GROUP = 64
DMA_CHUNK = 64
F32 = mybir.dt.float32
F16 = mybir.dt.float16


def _build_program(kp):
    nc = bacc.Bacc(None, target_bir_lowering=False)

    xh = nc.dram_tensor("xh", [128, RPC], F16, kind="ExternalInput")
    mt = nc.dram_tensor("mt", [128, K], F16, kind="ExternalInput")
    y = nc.dram_tensor("y", [128, TILES], F32, kind="ExternalOutput")

    with tile.TileContext(nc) as tc:
        with (
            tc.tile_pool(name="singles", bufs=1) as singles,
            tc.tile_pool(name="xin", bufs=6) as xin,
            tc.tile_pool(name="zp", bufs=3, space="PSUM") as zpool,
            tc.tile_pool(name="work", bufs=4) as work,
            tc.tile_pool(name="yout", bufs=1) as yout,
        ):
            sizes = []
            rem = TILES
            while rem > DMA_CHUNK:
                sizes.append(DMA_CHUNK)
                rem -= DMA_CHUNK
            while rem > 16:
                sizes.append(16)
                rem -= 16
            sizes.append(rem)

            # mt rides the same sync queue as the stream, issued first,
            # so the x torrent can't starve it (v4 lost 5us to that).
            mt_sb = singles.tile([128, K], F16)
            nc.sync.dma_start(mt_sb, mt[:, :])

            chunk_tiles = []
            c0 = 0
            for i, nct in enumerate(sizes):
                xh_t = xin.tile([128, DMA_CHUNK * 128], F16, tag="xh")
                nc.sync.dma_start(
                    xh_t[:, : nct * 128], xh[:, c0 * 128 : (c0 + nct) * 128]
                )
                chunk_tiles.append((c0, nct, xh_t))
                c0 += nct
            assert c0 == TILES

            y_sb = yout.tile([128, TILES], F32)

            stored = 0
            reduced = 0
            done_groups = 0

            def emit_ystore():
                nonlocal stored
                if reduced > stored:
                    nc.gpsimd.dma_start(
                        y[:, stored:reduced], y_sb[:, stored:reduced]
                    )
                    stored = reduced

            for c0, nct, xh_t in chunk_tiles:
                g0 = 0
                while g0 < nct:
                    nt = min(GROUP, nct - g0)
                    t0 = c0 + g0
                    zp = zpool.tile([128, GROUP, K], F32, tag="z")
                    for t in range(nt):
                        sl = slice((g0 + t) * 128, (g0 + t + 1) * 128)
                        nc.tensor.matmul(
                            zp[:, t, :], xh_t[:, sl], mt_sb[:, :],
                            start=True, stop=True,
                        )
                    elem = work.tile([128, GROUP, K], F16, tag="elem")
                    nc.scalar.activation(
                        elem[:, :nt, :], zp[:, :nt, :],
                        mybir.ActivationFunctionType.Relu,
                    )
                    ysl = y_sb[:, t0 : t0 + nt]
                    if kp > 0:
                        nc.vector.tensor_reduce(
                            ysl, elem[:, :nt, :kp],
                            axis=mybir.AxisListType.X, op=mybir.AluOpType.add,
                        )
                    if kp < K:
                        rn = work.tile([128, GROUP], F32, tag="rn")
                        nc.vector.tensor_reduce(
                            rn[:, :nt], elem[:, :nt, kp:],
                            axis=mybir.AxisListType.X, op=mybir.AluOpType.add,
                        )
                        if kp > 0:
                            nc.vector.tensor_tensor(
                                ysl, ysl, rn[:, :nt],
                                op=mybir.AluOpType.subtract,
                            )
                        else:
                            nc.vector.tensor_scalar(
                                ysl, rn[:, :nt], -1.0,
                                op=mybir.AluOpType.mult,
                            )
                    reduced = t0 + nt
                    done_groups += 1
                    if done_groups % 2 == 0:
                        emit_ystore()
                    g0 += nt

            emit_ystore()

    nc.compile()
    return nc


_NC_CACHE = {}


def _get_program(kp):
    if kp not in _NC_CACHE:
        _NC_CACHE[kp] = _build_program(kp)
    return _NC_CACHE[kp]


def _host_prep(x1, x2, V, W, b, U):
    x1 = np.asarray(x1, dtype=np.float32)
    x2 = np.asarray(x2, dtype=np.float64)
    V = np.asarray(V, dtype=np.float64)
    W = np.asarray(W, dtype=np.float64)
    b = np.asarray(b, dtype=np.float64)
    U = np.asarray(U, dtype=np.float64)

    M = V[:, :D] + np.einsum("kde,e->kd", W, x2[0])   # (K, D)
    c = (x2[0] @ V[:, D:].T) + b                      # (K,)
    u = U[:, 0]                                       # (K,)

    perm = np.argsort(u <= 0, kind="stable")
    kp = int(np.sum(u > 0))
    up = np.abs(u[perm])
    M2 = (M[perm] * up[:, None]).T                    # (D, K)
    c2 = c[perm] * up                                 # (K,)
    delta = np.linalg.pinv(M2.T) @ c2                 # (D,) min-norm offset

    mt = np.ascontiguousarray(M2.astype(np.float16))

    in_maps = []
    for cidx in range(NCORES):
        sl = x1[cidx * ROWS_PER_CORE : (cidx + 1) * ROWS_PER_CORE]
        hbuf = np.zeros((128, RPC), dtype=np.float16)
        hbuf[:, :ROWS_PER_CORE] = (
            sl.T + delta[:, None].astype(np.float32)
        ).astype(np.float16)
        in_maps.append({"xh": hbuf, "mt": mt})
    return in_maps, kp


def _gather(results):
    outs = []
    for cidx in range(NCORES):
        yc = np.asarray(results[cidx]["y"])
        outs.append(yc.T.reshape(-1)[:ROWS_PER_CORE])
    return np.concatenate(outs).reshape(N, 1).astype(np.float32)


def run_device(in_maps, kp, trace=False):
    from concourse.bass_utils import run_bass_kernel_spmd

    nc = _get_program(kp)
    res = run_bass_kernel_spmd(
        nc, in_maps, core_ids=list(range(NCORES)), trace=trace
    )
    return res


def kernel(x1, x2, V, W, b, U):
    in_maps, kp = _host_prep(x1, x2, V, W, b, U)
    res = run_device(in_maps, kp, trace=False)
    return _gather(res.results)


# revision 11
# speedup vs baseline: 1.0493x; 1.0024x over previous
"""NTN kernel: f16 stream with the bias folded into the input encoding.

y = relu(x1 @ M + c) @ u,  M = V[:,:D] + (W @ x2)^T  (128x16),
c = x2 @ V[:,D:]^T + b,    u = U[:,0].

Three host-side folds make the device program trivial:

1. u into M:   u_k relu(z_k + c_k) = s_k relu(|u_k| z_k + |u_k| c_k),
   so M2 = M diag|u| with positive-u columns permuted first (KP of
   them) and y = sum(first KP) - sum(rest).
2. c into x:   pick the min-norm row offset D with D @ M2 = c2
   (pinv of the well-conditioned random 128x16 M2; ||D|| ~ 0.3).
   Streaming x~ = x1 + D makes the matmul produce z + c directly --
   no bias matmul, no threshold tables on device.
3. f16 encode: x~ and M2 stream as f16 (2 B/elem -> 16 MB/core,
   rel err ~5e-4 vs tolerance 2e-2).

Device, per 64-tile group (one DMA chunk):
    PE:   one 128x128 @ 128x16 f16 matmul per row-tile into PSUM
    ACT:  relu straight out of PSUM into an f16 elem tile (also
          releases the PSUM bank; ACT is otherwise idle)
    DVE:  plain-sum reduce over first KP columns -> y, reduce over
          the rest -> rn, y -= rn   (f16 input = 2x DVE throughput)
    Pool: y stored in 2-group slabs
    SP:   mt param DMA first (so it can't be starved by the stream),
          then all x chunk DMAs on the single sync queue.

Engine loads per 2.44us group cadence: PE 1.7us, ACT 1.2, DVE 1.0,
Pool 0.4 -- nothing rate-limits but the DMA stream itself.  KP depends
on the sign pattern of u, so the program is built after inputs arrive
(cached per KP).  The final partial chunk is split small so almost
nothing trails the last DMA byte.
"""

import numpy as np

import concourse.bass as bass
import concourse.bacc as bacc
import concourse.mybir as mybir
import concourse.tile as tile

N, D, K = 500000, 128, 16
NCORES = 8
ROWS_PER_CORE = N // NCORES
TILES = 489
RPC = TILES * 128
GROUP = 64
DMA_CHUNK = 64
F32 = mybir.dt.float32
F16 = mybir.dt.float16


def _build_program(kp):
    nc = bacc.Bacc(None, target_bir_lowering=False)

    xh = nc.dram_tensor("xh", [128, RPC], F16, kind="ExternalInput")
    mt = nc.dram_tensor("mt", [128, K], F16, kind="ExternalInput")
    y = nc.dram_tensor("y", [128, TILES], F32, kind="ExternalOutput")

    with tile.TileContext(nc) as tc:
        with (
            tc.tile_pool(name="singles", bufs=1) as singles,
            tc.tile_pool(name="xin", bufs=6) as xin,
            tc.tile_pool(name="zp", bufs=3, space="PSUM") as zpool,
            tc.tile_pool(name="work", bufs=4) as work,
            tc.tile_pool(name="yout", bufs=4) as yout,
        ):
            sizes = []
            rem = TILES
            while rem > DMA_CHUNK:
                sizes.append(DMA_CHUNK)
                rem -= DMA_CHUNK
            while rem > 16:
                sizes.append(16)
                rem -= 16
            sizes.append(rem)

            # mt rides the same sync queue as the stream, issued first,
            # so the x torrent can't starve it (v4 lost 5us to that).
            mt_sb = singles.tile([128, K], F16)
            nc.sync.dma_start(mt_sb, mt[:, :])

            chunk_tiles = []
            c0 = 0
            for i, nct in enumerate(sizes):
                xh_t = xin.tile([128, DMA_CHUNK * 128], F16, tag="xh")
                nc.sync.dma_start(
                    xh_t[:, : nct * 128], xh[:, c0 * 128 : (c0 + nct) * 128]
                )
                chunk_tiles.append((c0, nct, xh_t))
                c0 += nct
            assert c0 == TILES

            # y slabs are double-buffered pool tiles (2 groups each), so
            # the Pool-engine store of slab i never WAR-blocks the DVE
            # reduces of slab i+1 (a single shared y buffer serialized
            # the whole pipeline through whole-tile dependency tracking).
            SLAB = 2 * GROUP
            yslab = None
            slab0 = 0          # first tile of the current slab
            reduced = 0        # tiles reduced so far

            def flush_slab():
                nonlocal yslab, slab0
                if yslab is not None and reduced > slab0:
                    nc.gpsimd.dma_start(
                        y[:, slab0:reduced], yslab[:, : reduced - slab0]
                    )
                yslab = None
                slab0 = reduced

            for c0, nct, xh_t in chunk_tiles:
                g0 = 0
                while g0 < nct:
                    nt = min(GROUP, nct - g0)
                    t0 = c0 + g0
                    zp = zpool.tile([128, GROUP, K], F32, tag="z")
                    for t in range(nt):
                        sl = slice((g0 + t) * 128, (g0 + t + 1) * 128)
                        nc.tensor.matmul(
                            zp[:, t, :], xh_t[:, sl], mt_sb[:, :],
                            start=True, stop=True,
                        )
                    elem = work.tile([128, GROUP, K], F16, tag="elem")
                    nc.scalar.activation(
                        elem[:, :nt, :], zp[:, :nt, :],
                        mybir.ActivationFunctionType.Relu,
                    )
                    if yslab is None:
                        yslab = yout.tile([128, SLAB], F32, tag="yslab")
                    ysl = yslab[:, t0 - slab0 : t0 - slab0 + nt]
                    if kp > 0:
                        nc.vector.tensor_reduce(
                            ysl, elem[:, :nt, :kp],
                            axis=mybir.AxisListType.X, op=mybir.AluOpType.add,
                        )
                    if kp < K:
                        rn = work.tile([128, GROUP], F32, tag="rn")
                        nc.vector.tensor_reduce(
                            rn[:, :nt], elem[:, :nt, kp:],
                            axis=mybir.AxisListType.X, op=mybir.AluOpType.add,
                        )
                        if kp > 0:
                            nc.vector.tensor_tensor(
                                ysl, ysl, rn[:, :nt],
                                op=mybir.AluOpType.subtract,
                            )
                        else:
                            nc.vector.tensor_scalar(
                                ysl, rn[:, :nt], -1.0,
                                op=mybir.AluOpType.mult,
                            )
                    reduced = t0 + nt
                    if reduced - slab0 >= SLAB:
                        flush_slab()
                    g0 += nt

            flush_slab()

    nc.compile()
    return nc


_NC_CACHE = {}


def _get_program(kp):
    if kp not in _NC_CACHE:
        _NC_CACHE[kp] = _build_program(kp)
    return _NC_CACHE[kp]


def _host_prep(x1, x2, V, W, b, U):
    x1 = np.asarray(x1, dtype=np.float32)
    x2 = np.asarray(x2, dtype=np.float64)
    V = np.asarray(V, dtype=np.float64)
    W = np.asarray(W, dtype=np.float64)
    b = np.asarray(b, dtype=np.float64)
    U = np.asarray(U, dtype=np.float64)

    M = V[:, :D] + np.einsum("kde,e->kd", W, x2[0])   # (K, D)
    c = (x2[0] @ V[:, D:].T) + b                      # (K,)
    u = U[:, 0]                                       # (K,)

    perm = np.argsort(u <= 0, kind="stable")
    kp = int(np.sum(u > 0))
    up = np.abs(u[perm])
    M2 = (M[perm] * up[:, None]).T                    # (D, K)
    c2 = c[perm] * up                                 # (K,)
    delta = np.linalg.pinv(M2.T) @ c2                 # (D,) min-norm offset

    mt = np.ascontiguousarray(M2.astype(np.float16))

    in_maps = []
    for cidx in range(NCORES):
        sl = x1[cidx * ROWS_PER_CORE : (cidx + 1) * ROWS_PER_CORE]
        hbuf = np.zeros((128, RPC), dtype=np.float16)
        hbuf[:, :ROWS_PER_CORE] = (
            sl.T + delta[:, None].astype(np.float32)
        ).astype(np.float16)
        in_maps.append({"xh": hbuf, "mt": mt})
    return in_maps, kp


def _gather(results):
    outs = []
    for cidx in range(NCORES):
        yc = np.asarray(results[cidx]["y"])
        outs.append(yc.T.reshape(-1)[:ROWS_PER_CORE])
    return np.concatenate(outs).reshape(N, 1).astype(np.float32)


def run_device(in_maps, kp, trace=False):
    from concourse.bass_utils import run_bass_kernel_spmd

    nc = _get_program(kp)
    res = run_bass_kernel_spmd(
        nc, in_maps, core_ids=list(range(NCORES)), trace=trace
    )
    return res


def kernel(x1, x2, V, W, b, U):
    in_maps, kp = _host_prep(x1, x2, V, W, b, U)
    res = run_device(in_maps, kp, trace=False)
    return _gather(res.results)
